# revision 1
# baseline (speedup 1.0000x reference)
"""Trainium2 Bass kernel for nn_Downsampler: depthwise 4x4 conv, stride 4,
VALID padding, one shared (runtime) 4x4 kernel across all channels.

  x: (16, 8, 1024, 1024) f32, kernel: (4, 4) f32 -> out: (16, 8, 256, 256) f32

Sharding: pure data parallel over batch N=16 -> 2 batches per core on 8 cores.

Math: out[o, j] = sum_{di,dj} k[di,dj] * x[4o+di, 4j+dj], rows flattened over
(n, c, h) since every image row has W=1024 and slabs never straddle an (n, c)
boundary (1024 rows per image, slab = 512 rows).

Implementation (per core, R = 2*8*1024 = 16384 rows): per slab of 512 input
rows one SBUF tile [128, 4096] holds the rows interleaved by 128 (partition p,
quarter d -> row 512*s + 128*d + p). The whole conv is 8 accumulating
matmuls per slab on the TensorEngine with a host-built banded weight matrix:

  KW[p, dj*32 + m] = kernel[p % 4, dj] * (p // 4 == m)        (128 x 128)
  psum_b[m, (dd, j)] += sum_p KW[p, dj*32+m] * xt[p, (2b+dd)*1024 + 4*j + dj]

PSUM accumulates over dj (start/stop flags); each psum bank [32, 512] covers
64 output rows, evicted PSUM->SBUF on ScalarE, then DMA'd to DRAM.
"""

import json
from contextlib import ExitStack

import numpy as np

import concourse.bass as bass
import concourse.mybir as mybir
from concourse.tile import TileContext
from concourse.bass_utils import run_bass_kernel_spmd

N, C, H, W = 16, 8, 1024, 1024
F = 4
N_CORES = 8
R = (N // N_CORES) * C * H  # input rows per core (16384)
RO = R // F  # output rows per core (4096)
WO = W // F  # output row length (256)


def _split_excess_waits(bir_bytes: bytes, max_waits: int = 1) -> bytes:
    """The public neuronxcc walrus supports at most ONE sync wait per
    instruction; hoist excess waits onto NoOps inserted just before."""
    m = json.loads(bir_bytes)

    def fix(blocks):
        for bb in blocks:
            out = []
            for ins in bb.get("instructions", []):
                si = ins.get("sync_info")
                waits = (si or {}).get("on_wait") or []
                if len(waits) > max_waits:
                    extra = waits[:-max_waits]
                    si["on_wait"] = waits[-max_waits:]
                    for i in range(0, len(extra), max_waits):
                        out.append(
                            {
                                "debug": ins.get("debug", 0),
                                "engine": ins["engine"],
                                "ins": [],
                                "outs": [],
                                "name": f"{ins['name']}-ws{i}",
                                "opcode": "NoOp",
                                "sync_info": {
                                    "on_update": [],
                                    "on_wait": extra[i : i + max_waits],
                                },
                            }
                        )
                out.append(ins)
            bb["instructions"] = out
            fix(bb.get("blocks", []))

    for f in m["functions"]:
        fix(f["blocks"])
    return json.dumps(m).encode()


def _make_kw(kernel: np.ndarray) -> np.ndarray:
    """Banded matmul weights [128, 128] built on host from the 4x4 kernel."""
    kernel = np.asarray(kernel, dtype=np.float32)
    assert kernel.shape == (F, F)
    kw = np.zeros((128, 128), dtype=np.float32)
    p = np.arange(128)
    for dj in range(F):
        kw[p, dj * 32 + p // 4] = kernel[p % 4, dj]
    return kw


def _build_nc(rows: int, xt_bufs: int = 3, psum_bufs: int = 4) -> bass.Bass:
    assert rows % 512 == 0
    n_slabs = rows // 512

    nc = bass.Bass("TRN2", target_bir_lowering=False, debug=False)
    x = nc.dram_tensor("x", [rows, W], mybir.dt.float32, kind="ExternalInput")
    kw = nc.dram_tensor("kw", [128, 128], mybir.dt.float32, kind="ExternalInput")
    y = nc.dram_tensor("y", [rows // F, WO], mybir.dt.float32, kind="ExternalOutput")

    with TileContext(nc) as tc:
        with ExitStack() as ctx:
            kw_pool = ctx.enter_context(tc.tile_pool(name="kw_pool", bufs=1))
            kwt = kw_pool.tile([128, 128], mybir.dt.float32)
            nc.sync.dma_start(kwt[:], kw.ap())

            x_pool = ctx.enter_context(tc.tile_pool(name="x_pool", bufs=xt_bufs))
            ps_pool = ctx.enter_context(
                tc.tile_pool(name="ps_pool", bufs=psum_bufs, space="PSUM")
            )
            o_pool = ctx.enter_context(tc.tile_pool(name="o_pool", bufs=4))

            for s in range(n_slabs):
                xt = x_pool.tile([128, 4 * W], mybir.dt.float32)
                src = x.ap()[s * 512 : (s + 1) * 512, :].rearrange(
                    "(d p) w -> p d w", p=128
                )
                nc.sync.dma_start(xt[:].rearrange("p (d w) -> p d w", d=4), src)

                # [128, d, j, q]: element (p, d, j, q) = xt[p, d*W + 4j + q]
                xv = xt[:].rearrange("p (d j q) -> p d j q", d=4, q=F)

                for b in range(2):
                    pt = ps_pool.tile([32, 512], mybir.dt.float32)
                    for dj in range(F):
                        nc.tensor.matmul(
                            pt[:],
                            kwt[:, dj * 32 : (dj + 1) * 32],
                            xv[:, 2 * b : 2 * b + 2, :, dj],
                            start=(dj == 0),
                            stop=(dj == F - 1),
                        )
                    # evict PSUM -> SBUF (DMA cannot read PSUM here)
                    ot = o_pool.tile([32, 512], mybir.dt.float32)
                    nc.scalar.copy(ot[:], pt[:])
                    # sbuf [32, (dd, j)] -> y rows s*128 + 64b + 32*dd + m
                    base = s * 128 + 64 * b
                    dst = y.ap()[base : base + 64, :].rearrange(
                        "(dd m) j -> m dd j", dd=2
                    )
                    nc.sync.dma_start(
                        dst, ot[:].rearrange("m (dd j) -> m dd j", dd=2)
                    )

    # walrus 1-wait-per-instruction workaround, applied at serialization time
    orig = nc.to_json_bytes
    nc.to_json_bytes = lambda: _split_excess_waits(orig())
    return nc


_NC_CACHE: dict[int, bass.Bass] = {}


def _get_nc(rows: int = R) -> bass.Bass:
    if rows not in _NC_CACHE:
        _NC_CACHE[rows] = _build_nc(rows)
    return _NC_CACHE[rows]


def run_spmd(x: np.ndarray, kern: np.ndarray, **spmd_kwargs):
    """Shard, run on 8 cores, gather. Returns (output, BassKernelResults)."""
    assert x.shape == (N, C, H, W) and kern.shape == (F, F)
    x = np.ascontiguousarray(x, dtype=np.float32)
    kw = _make_kw(kern)
    nb = N // N_CORES
    in_maps = [
        {"x": x[i * nb : (i + 1) * nb].reshape(R, W), "kw": kw}
        for i in range(N_CORES)
    ]
    nc = _get_nc()
    res = run_bass_kernel_spmd(nc, in_maps, core_ids=list(range(N_CORES)), **spmd_kwargs)
    out = np.concatenate(
        [res.results[i]["y"].reshape(nb, C, H // F, WO) for i in range(N_CORES)],
        axis=0,
    )
    return out, res


def kernel(x: np.ndarray, kernel: np.ndarray) -> np.ndarray:
    out, _ = run_spmd(x, kernel)
    return out


# revision 3
# speedup vs baseline: 1.2758x; 1.2758x over previous
"""Trainium2 Bass kernel for nn_Downsampler: depthwise 4x4 conv, stride 4,
VALID padding, one shared (runtime) 4x4 kernel across all channels.

  x: (16, 8, 1024, 1024) f32, kernel: (4, 4) f32 -> out: (16, 8, 256, 256) f32

Sharding: pure data parallel over batch N=16 -> 2 batches per core on 8 cores.

Math: out[o, j] = sum_{di,dj} k[di,dj] * x[4o+di, 4j+dj], rows flattened over
(n, c, h) since every image row has W=1024 and slabs never straddle an (n, c)
boundary (1024 rows per image, slab = 512 rows).

Two-stage implementation, per slab of 512 input rows held as an SBUF tile
[128, 4096] (partition p, quarter d -> row 512*s + 128*d + p):

1. Horizontal pass (W-downsample). Row r uses kernel row k[r%4, :], and
   r%4 == p%4 in every quarter, so the weights are a per-partition scalar
   ks[p, dj] = kernel[p%4, dj]:
       hp[p, (d, j)] = sum_dj ks[p, dj] * xt[p, (d, 4j+dj)]
   4 fused multiply-accumulates spread across engines: ScalarE
   ACTIVATE(Copy, scale) for dj=0, GpSimd STT for dj=1, VectorE STT for
   dj=2,3 (strided fp32 reads run at 1x on these engines regardless).

2. Vertical pass (H-downsample) on the TensorEngine with a 0/1 selection
   matrix sel[p, m] = (p//4 == m), contracting the 4 rows of each group:
       psum[m, (d, j)] = sum_p sel[p, m] * hp[p, (d, j)]
   Dense fp32 rhs (1 col/4cyc), N=512 per matmul; output row = 32*d + m.

PSUM is evicted to SBUF on ScalarE, then DMA'd to DRAM. All arithmetic is
fp32, so the result matches the f32 jax reference to rounding (~1e-7).
"""

import json
from contextlib import ExitStack

import numpy as np

import concourse.bass as bass
import concourse.mybir as mybir
from concourse.tile import TileContext
from concourse.bass_utils import run_bass_kernel_spmd

N, C, H, W = 16, 8, 1024, 1024
F = 4
N_CORES = 8
R = (N // N_CORES) * C * H  # input rows per core (16384)
WO = W // F  # output row length (256)


def _split_excess_waits(bir_bytes: bytes, max_waits: int = 1) -> bytes:
    """The public neuronxcc walrus supports at most ONE sync wait per
    instruction; hoist excess waits onto NoOps inserted just before."""
    m = json.loads(bir_bytes)

    def fix(blocks):
        for bb in blocks:
            out = []
            for ins in bb.get("instructions", []):
                si = ins.get("sync_info")
                waits = (si or {}).get("on_wait") or []
                if len(waits) > max_waits:
                    extra = waits[:-max_waits]
                    si["on_wait"] = waits[-max_waits:]
                    for i in range(0, len(extra), max_waits):
                        out.append(
                            {
                                "debug": ins.get("debug", 0),
                                "engine": ins["engine"],
                                "ins": [],
                                "outs": [],
                                "name": f"{ins['name']}-ws{i}",
                                "opcode": "NoOp",
                                "sync_info": {
                                    "on_update": [],
                                    "on_wait": extra[i : i + max_waits],
                                },
                            }
                        )
                out.append(ins)
            bb["instructions"] = out
            fix(bb.get("blocks", []))

    for f in m["functions"]:
        fix(f["blocks"])
    return json.dumps(m).encode()


def _make_ks(kernel: np.ndarray) -> np.ndarray:
    """Per-partition horizontal weights [128, 4]: ks[p, dj] = kernel[p%4, dj]."""
    kernel = np.asarray(kernel, dtype=np.float32)
    assert kernel.shape == (F, F)
    return np.ascontiguousarray(kernel[np.arange(128) % F, :])


def _make_sel() -> np.ndarray:
    """Vertical selection matmul weights [128, 32]: sel[p, m] = (p//4 == m)."""
    p = np.arange(128)
    return (p[:, None] // F == np.arange(32)[None, :]).astype(np.float32)


def _build_nc(
    rows: int, xt_bufs: int = 3, hp_bufs: int = 3, psum_bufs: int = 4, o_bufs: int = 4
) -> bass.Bass:
    assert rows % 512 == 0
    n_slabs = rows // 512

    nc = bass.Bass("TRN2", target_bir_lowering=False, debug=False)
    x = nc.dram_tensor("x", [rows, W], mybir.dt.float32, kind="ExternalInput")
    ks = nc.dram_tensor("ks", [128, F], mybir.dt.float32, kind="ExternalInput")
    sel = nc.dram_tensor("sel", [128, 32], mybir.dt.float32, kind="ExternalInput")
    y = nc.dram_tensor("y", [rows // F, WO], mybir.dt.float32, kind="ExternalOutput")

    mult = mybir.AluOpType.mult
    add = mybir.AluOpType.add

    with TileContext(nc) as tc:
        with ExitStack() as ctx:
            const_pool = ctx.enter_context(tc.tile_pool(name="const_pool", bufs=1))
            kst = const_pool.tile([128, F], mybir.dt.float32)
            nc.sync.dma_start(kst[:], ks.ap())
            selt = const_pool.tile([128, 32], mybir.dt.float32)
            nc.sync.dma_start(selt[:], sel.ap())

            x_pool = ctx.enter_context(tc.tile_pool(name="x_pool", bufs=xt_bufs))
            hp_pool = ctx.enter_context(tc.tile_pool(name="hp_pool", bufs=hp_bufs))
            ps_pool = ctx.enter_context(
                tc.tile_pool(name="ps_pool", bufs=psum_bufs, space="PSUM")
            )
            o_pool = ctx.enter_context(tc.tile_pool(name="o_pool", bufs=o_bufs))

            for s in range(n_slabs):
                xt = x_pool.tile([128, 4 * W], mybir.dt.float32)
                src = x.ap()[s * 512 : (s + 1) * 512, :].rearrange(
                    "(d p) w -> p d w", p=128
                )
                nc.sync.dma_start(xt[:].rearrange("p (d w) -> p d w", d=4), src)

                # [128, d, j, q]: element (p, d, j, q) = xt[p, d*W + 4j + q]
                xv = xt[:].rearrange("p (d j q) -> p d j q", d=4, q=F)

                hp = hp_pool.tile([128, 4 * WO], mybir.dt.float32)
                hpv = hp[:].rearrange("p (d j) -> p d j", d=4)
                hp2 = hp_pool.tile([128, 4 * WO], mybir.dt.float32)
                hp2v = hp2[:].rearrange("p (d j) -> p d j", d=4)

                # hp = ks[:,0]*x(0), hp2 = ks[:,1]*x(1)  on ScalarE
                for dst, dj in ((hpv, 0), (hp2v, 1)):
                    nc.scalar.activation(
                        dst,
                        xv[:, :, :, dj],
                        mybir.ActivationFunctionType.Copy,
                        scale=kst[:, dj : dj + 1],
                    )
                # hp += hp2  on GpSimd (Pool has no TensorScalarPtr)
                nc.gpsimd.tensor_tensor(hp[:], hp[:], hp2[:], add)
                # hp += ks[:,dj] * x(dj)  on VectorE (dj=2,3)
                for dj in (2, 3):
                    nc.vector.scalar_tensor_tensor(
                        hpv, xv[:, :, :, dj], kst[:, dj : dj + 1], hpv, mult, add
                    )

                # vertical pass: psum[m, (d, j)] = sum_p sel[p, m] hp[p, (d, j)]
                pt = ps_pool.tile([32, 4 * WO], mybir.dt.float32)
                for c in range(2):
                    nc.tensor.matmul(
                        pt[:, c * 512 : (c + 1) * 512],
                        selt[:],
                        hp[:, c * 512 : (c + 1) * 512],
                        start=True,
                        stop=True,
                    )

                # evict PSUM -> SBUF (DMA cannot read PSUM here)
                ot = o_pool.tile([32, 4 * WO], mybir.dt.float32)
                nc.scalar.copy(ot[:], pt[:])

                # ot[m, (d, j)] -> y row s*128 + 32*d + m
                base = s * 128
                dst = y.ap()[base : base + 128, :].rearrange("(d m) j -> m d j", d=4)
                nc.sync.dma_start(dst, ot[:].rearrange("m (d j) -> m d j", d=4))

    # walrus 1-wait-per-instruction workaround, applied at serialization time
    orig = nc.to_json_bytes
    nc.to_json_bytes = lambda: _split_excess_waits(orig())
    return nc


_NC_CACHE: dict[int, bass.Bass] = {}


def _get_nc(rows: int = R) -> bass.Bass:
    if rows not in _NC_CACHE:
        _NC_CACHE[rows] = _build_nc(rows)
    return _NC_CACHE[rows]


def run_spmd(x: np.ndarray, kern: np.ndarray, **spmd_kwargs):
    """Shard, run on 8 cores, gather. Returns (output, BassKernelResults)."""
    assert x.shape == (N, C, H, W) and kern.shape == (F, F)
    x = np.ascontiguousarray(x, dtype=np.float32)
    ks = _make_ks(kern)
    sel = _make_sel()
    nb = N // N_CORES
    in_maps = [
        {"x": x[i * nb : (i + 1) * nb].reshape(R, W), "ks": ks, "sel": sel}
        for i in range(N_CORES)
    ]
    nc = _get_nc()
    res = run_bass_kernel_spmd(
        nc, in_maps, core_ids=list(range(N_CORES)), **spmd_kwargs
    )
    out = np.concatenate(
        [res.results[i]["y"].reshape(nb, C, H // F, WO) for i in range(N_CORES)],
        axis=0,
    )
    return out, res


def kernel(x: np.ndarray, kernel: np.ndarray) -> np.ndarray:
    out, _ = run_spmd(x, kernel)
    return out


# revision 6
# speedup vs baseline: 1.4057x; 1.1019x over previous
"""Trainium2 Bass kernel for nn_Downsampler: depthwise 4x4 conv, stride 4,
VALID padding, one shared (runtime) 4x4 kernel across all channels.

  x: (16, 8, 1024, 1024) f32, kernel: (4, 4) f32 -> out: (16, 8, 256, 256) f32

Sharding: pure data parallel over batch N=16 -> 2 batches per core on 8 cores.

Math: out[o, j] = sum_{di,dj} k[di,dj] * x[4o+di, 4j+dj], rows flattened over
(n, c, h) since every image row has W=1024 and slabs never straddle an (n, c)
boundary (1024 rows per image, slab = 512 rows).

Two-stage implementation, per slab of 512 input rows held as an SBUF tile
[128, 4096] (partition p, quarter d -> row 512*s + 128*d + p):

1. Horizontal pass (W-downsample). Row r uses kernel row k[r%4, :], and
   r%4 == p%4 in every quarter, so the weights are a per-partition scalar
   ks[p, dj] = kernel[p%4, dj]:
       hp[p, (d, j)] = sum_dj ks[p, dj] * xt[p, (d, 4j+dj)]
   4 fused multiply-accumulates spread across engines: ScalarE
   ACTIVATE(Copy, scale) for dj=0, GpSimd STT for dj=1, VectorE STT for
   dj=2,3 (strided fp32 reads run at 1x on these engines regardless).

2. Vertical pass (H-downsample) on the TensorEngine with a 0/1 selection
   matrix sel[p, m] = (p//4 == m), contracting the 4 rows of each group:
       psum[m, (d, j)] = sum_p sel[p, m] * hp[p, (d, j)]
   Dense fp32 rhs (1 col/4cyc), N=512 per matmul; output row = 32*d + m.

PSUM is evicted to SBUF on ScalarE, then DMA'd to DRAM. All arithmetic is
fp32, so the result matches the f32 jax reference to rounding (~1e-7).
"""

import json
from contextlib import ExitStack

import numpy as np

import concourse.bass as bass
import concourse.mybir as mybir
from concourse.tile import TileContext
from concourse.bass_utils import run_bass_kernel_spmd

N, C, H, W = 16, 8, 1024, 1024
F = 4
N_CORES = 8
R = (N // N_CORES) * C * H  # input rows per core (16384)
WO = W // F  # output row length (256)


def _split_excess_waits(bir_bytes: bytes, max_waits: int = 1) -> bytes:
    """The public neuronxcc walrus supports at most ONE sync wait per
    instruction; hoist excess waits onto NoOps inserted just before."""
    m = json.loads(bir_bytes)

    def fix(blocks):
        for bb in blocks:
            out = []
            for ins in bb.get("instructions", []):
                si = ins.get("sync_info")
                waits = (si or {}).get("on_wait") or []
                if len(waits) > max_waits:
                    extra = waits[:-max_waits]
                    si["on_wait"] = waits[-max_waits:]
                    for i in range(0, len(extra), max_waits):
                        out.append(
                            {
                                "debug": ins.get("debug", 0),
                                "engine": ins["engine"],
                                "ins": [],
                                "outs": [],
                                "name": f"{ins['name']}-ws{i}",
                                "opcode": "NoOp",
                                "sync_info": {
                                    "on_update": [],
                                    "on_wait": extra[i : i + max_waits],
                                },
                            }
                        )
                out.append(ins)
            bb["instructions"] = out
            fix(bb.get("blocks", []))

    for f in m["functions"]:
        fix(f["blocks"])
    return json.dumps(m).encode()


def _make_ks(kernel: np.ndarray) -> np.ndarray:
    """Per-partition horizontal weights [128, 4]: ks[p, dj] = kernel[p%4, dj]."""
    kernel = np.asarray(kernel, dtype=np.float32)
    assert kernel.shape == (F, F)
    return np.ascontiguousarray(kernel[np.arange(128) % F, :])


def _make_sel() -> np.ndarray:
    """Vertical selection matmul weights [128, 32]: sel[p, m] = (p//4 == m)."""
    p = np.arange(128)
    return (p[:, None] // F == np.arange(32)[None, :]).astype(np.float32)


def _build_nc(
    rows: int, xt_bufs: int = 5, hp_bufs: int = 3, psum_bufs: int = 4, o_bufs: int = 4
) -> bass.Bass:
    assert rows % 512 == 0
    n_slabs = rows // 512

    nc = bass.Bass("TRN2", target_bir_lowering=False, debug=False)
    x = nc.dram_tensor("x", [rows, W], mybir.dt.float32, kind="ExternalInput")
    ks = nc.dram_tensor("ks", [128, F], mybir.dt.float32, kind="ExternalInput")
    sel = nc.dram_tensor("sel", [128, 32], mybir.dt.float32, kind="ExternalInput")
    y = nc.dram_tensor("y", [rows // F, WO], mybir.dt.float32, kind="ExternalOutput")

    mult = mybir.AluOpType.mult
    add = mybir.AluOpType.add

    with TileContext(nc) as tc:
        with ExitStack() as ctx:
            const_pool = ctx.enter_context(tc.tile_pool(name="const_pool", bufs=1))
            kst = const_pool.tile([128, F], mybir.dt.float32)
            nc.sync.dma_start(kst[:], ks.ap())
            selt = const_pool.tile([128, 32], mybir.dt.float32)
            nc.sync.dma_start(selt[:], sel.ap())

            x_pool = ctx.enter_context(tc.tile_pool(name="x_pool", bufs=xt_bufs))
            hp_pool = ctx.enter_context(tc.tile_pool(name="hp_pool", bufs=hp_bufs))
            ps_pool = ctx.enter_context(
                tc.tile_pool(name="ps_pool", bufs=psum_bufs, space="PSUM")
            )
            o_pool = ctx.enter_context(tc.tile_pool(name="o_pool", bufs=o_bufs))

            for s in range(n_slabs):
                xt = x_pool.tile([128, 4 * W], mybir.dt.float32)
                src = x.ap()[s * 512 : (s + 1) * 512, :].rearrange(
                    "(d p) w -> p d w", p=128
                )
                nc.sync.dma_start(xt[:].rearrange("p (d w) -> p d w", d=4), src)

                # [128, d, j, q]: element (p, d, j, q) = xt[p, d*W + 4j + q]
                xv = xt[:].rearrange("p (d j q) -> p d j q", d=4, q=F)

                hp = hp_pool.tile([128, 4 * WO], mybir.dt.float32)
                hpv = hp[:].rearrange("p (d j) -> p d j", d=4)
                hp2 = hp_pool.tile([128, 4 * WO], mybir.dt.float32)
                hp2v = hp2[:].rearrange("p (d j) -> p d j", d=4)

                # hp = ks[:,0]*x(0)  on ScalarE
                nc.scalar.activation(
                    hpv,
                    xv[:, :, :, 0],
                    mybir.ActivationFunctionType.Copy,
                    scale=kst[:, 0:1],
                )
                # hp2 = ks[:,1]*x(1): alternate ScalarE / GpSimd by slab parity
                if s % 2 == 0:
                    nc.scalar.activation(
                        hp2v,
                        xv[:, :, :, 1],
                        mybir.ActivationFunctionType.Copy,
                        scale=kst[:, 1:2],
                    )
                else:
                    nc.gpsimd.tensor_tensor(
                        hp2v,
                        xv[:, :, :, 1],
                        kst[:, 1:2].broadcast_to([128, 4, WO]),
                        mult,
                    )
                # hp += hp2  on GpSimd (Pool has no TensorScalarPtr)
                nc.gpsimd.tensor_tensor(hp[:], hp[:], hp2[:], add)
                # hp += ks[:,dj] * x(dj)  on VectorE (dj=2,3)
                for dj in (2, 3):
                    nc.vector.scalar_tensor_tensor(
                        hpv, xv[:, :, :, dj], kst[:, dj : dj + 1], hpv, mult, add
                    )

                # vertical pass: psum[m, (d, j)] = sum_p sel[p, m] hp[p, (d, j)]
                pt = ps_pool.tile([32, 4 * WO], mybir.dt.float32)
                for c in range(2):
                    nc.tensor.matmul(
                        pt[:, c * 512 : (c + 1) * 512],
                        selt[:],
                        hp[:, c * 512 : (c + 1) * 512],
                        start=True,
                        stop=True,
                    )

                # evict PSUM -> SBUF (DMA cannot read PSUM here)
                ot = o_pool.tile([32, 4 * WO], mybir.dt.float32)
                nc.scalar.copy(ot[:], pt[:])

                # ot[m, (d, j)] -> y row s*128 + 32*d + m
                # output DMA rides the ScalarE HWDGE ring so it never
                # head-of-line-blocks the input stream on the SP ring
                base = s * 128
                dst = y.ap()[base : base + 128, :].rearrange("(d m) j -> m d j", d=4)
                nc.scalar.dma_start(dst, ot[:].rearrange("m (d j) -> m d j", d=4))

    # walrus 1-wait-per-instruction workaround, applied at serialization time
    orig = nc.to_json_bytes
    nc.to_json_bytes = lambda: _split_excess_waits(orig())
    return nc


_NC_CACHE: dict[int, bass.Bass] = {}


def _get_nc(rows: int = R) -> bass.Bass:
    if rows not in _NC_CACHE:
        _NC_CACHE[rows] = _build_nc(rows)
    return _NC_CACHE[rows]


def run_spmd(x: np.ndarray, kern: np.ndarray, **spmd_kwargs):
    """Shard, run on 8 cores, gather. Returns (output, BassKernelResults)."""
    assert x.shape == (N, C, H, W) and kern.shape == (F, F)
    x = np.ascontiguousarray(x, dtype=np.float32)
    ks = _make_ks(kern)
    sel = _make_sel()
    nb = N // N_CORES
    in_maps = [
        {"x": x[i * nb : (i + 1) * nb].reshape(R, W), "ks": ks, "sel": sel}
        for i in range(N_CORES)
    ]
    nc = _get_nc()
    res = run_bass_kernel_spmd(
        nc, in_maps, core_ids=list(range(N_CORES)), **spmd_kwargs
    )
    out = np.concatenate(
        [res.results[i]["y"].reshape(nb, C, H // F, WO) for i in range(N_CORES)],
        axis=0,
    )
    return out, res


def kernel(x: np.ndarray, kernel: np.ndarray) -> np.ndarray:
    out, _ = run_spmd(x, kernel)
    return out
